# revision 1
# baseline (speedup 1.0000x reference)
"""Trainium2 Bass kernel for DiffDock-style GAT model (8 NeuronCores, SPMD).

Strategy:
- Edges (+self-loops) sorted by destination; destinations tiled into 128-node
  windows; windows assigned round-robin to the 8 cores. Node table rows are
  laid out rank-major so a single AllGather publishes every core's shard.
- Per layer, each core builds its shard of a node table
  [x (128) | asn (4) | adn (4)] in bf16, AllGathers it, then processes its
  edge blocks: indirect-DMA gather of source rows (the critical path),
  indicator matmuls (built on-chip from dst-local ids via iota compares and a
  PE broadcast-transpose) for the edge-softmax scatter/spread ops, and one
  PSUM-accumulated matmul per block for message aggregation + softmax sums.
- Softmax uses no max-subtraction (validated: |e| < 2 for these inputs), so
  alpha normalization commutes with the scatter and is applied per window.
- Mean-pool via batch-indicator matmuls, AllReduce of [128,64] pooled
  features, then the small prediction MLP replicated on every core.
"""

import numpy as np
import ml_dtypes

HID = 128
HEADS = 4
DEPTH = 4
TDIM = 32
G = 32
P = 128
N_CORES = 8
ROW = 136  # table row: x(128) + asn(4) + adn(4)


# ----------------------------------------------------------------------------
# host-side helpers
# ----------------------------------------------------------------------------

def _np32(a):
    return np.asarray(a, dtype=np.float32)


def _time_embedding(t_w1, t_b1, t_w2, t_b2):
    # sinusoidal_emb(0) = [sin(0)*16, cos(0)*16] = [0]*16 + [1]*16
    te = np.concatenate([np.zeros(16, np.float32), np.ones(16, np.float32)])
    h = te @ _np32(t_w1).T + _np32(t_b1)
    h = h * (1.0 / (1.0 + np.exp(-h)))  # silu
    return h @ _np32(t_w2).T + _np32(t_b2)  # [128]


def _blockdiag(a):
    # a: [HEADS, HID//HEADS] -> [HID, HEADS] block diagonal
    m = np.zeros((HID, HEADS), np.float32)
    for h in range(HEADS):
        m[32 * h : 32 * (h + 1), h] = a[h]
    return m


class _Branch:
    """Host-side graph prep for one branch (protein or ligand).

    Self-loops are NOT added to the edge list; they are handled as a cheap
    contiguous "self block" per window (identity indicator, no gather).
    Windows are dealt snake-wise across cores by descending block count so
    per-slot maxima stay tight and cores stay balanced.
    """

    def __init__(self, x, edge_index, batch):
        N = x.shape[0]
        src = np.asarray(edge_index[0], np.int64)
        dst = np.asarray(edge_index[1], np.int64)
        order = np.argsort(dst, kind="stable")
        self.src = src[order]
        self.dst = dst[order]
        self.N = N
        self.batch = np.asarray(batch, np.int64)
        n_win = -(-N // P)
        n_win = -(-n_win // N_CORES) * N_CORES  # multiple of 8
        self.n_win = n_win
        self.wc = n_win // N_CORES  # windows per core
        win = self.dst // P
        wcnt = np.bincount(win, minlength=n_win)
        self.wcnt = wcnt
        self.wstart = np.concatenate([[0], np.cumsum(wcnt)])
        # snake-deal windows (desc by block count) across cores
        blocks = -(-wcnt // P)
        rank = np.argsort(-blocks, kind="stable")
        self.win_of = np.zeros((N_CORES, self.wc), np.int64)  # [core, slot] -> w
        core_of = np.zeros(n_win, np.int64)
        slot_of = np.zeros(n_win, np.int64)
        for s in range(self.wc):
            grp = rank[s * N_CORES : (s + 1) * N_CORES]
            cores = range(N_CORES) if s % 2 == 0 else range(N_CORES - 1, -1, -1)
            for c, w in zip(cores, grp):
                self.win_of[c, s] = w
                core_of[w] = c
                slot_of[w] = s
        self.core_of, self.slot_of = core_of, slot_of
        # per-slot block count = max over cores (shared SPMD program shape)
        self.nbl = [
            int(max(blocks[self.win_of[c, s]] for c in range(N_CORES)))
            for s in range(self.wc)
        ]

    def shard_slot(self, node):
        """(core, row_within_branch_shard) for each node id."""
        w = node // P
        return self.core_of[w], self.slot_of[w] * P + node % P


def _prep(inputs):
    pb = _Branch(inputs["protein_x"], inputs["protein_edge_index"], inputs["protein_batch"])
    lb = _Branch(inputs["ligand_x"], inputs["ligand_edge_index"], inputs["ligand_batch"])
    meta = {
        "p_wc": pb.wc, "l_wc": lb.wc,
        "p_nbl": pb.nbl, "l_nbl": lb.nbl,
    }

    def table_row(branch, node):
        # branch tables are separate; rows are rank-major
        c, r = branch.shard_slot(node)
        return c * branch.wc * P + r

    te = _time_embedding(inputs["t_w1"], inputs["t_b1"], inputs["t_w2"], inputs["t_b2"])

    per_core = [dict() for _ in range(N_CORES)]
    shared = {}

    for tag, br in (("p", pb), ("l", lb)):
        wc = br.wc
        btot = sum(br.nbl)
        cbase = np.concatenate([[0], np.cumsum(br.nbl)]).astype(np.int64)
        # global row index of each edge's src in the full table
        src_rows = table_row(br, br.src)
        xin = _np32(inputs[f"{'protein' if tag == 'p' else 'ligand'}_x"])
        for c in range(N_CORES):
            gidx = np.zeros((P, btot), np.int32)
            dstloc = np.full((P, btot), -1.0, np.float32)
            ind = np.zeros((P, wc * G), np.float32)
            xt = np.zeros((16, wc * P), np.float32)
            for s in range(wc):
                w = int(br.win_of[c, s])
                e0, e1 = br.wstart[w], br.wstart[w + 1]
                for b in range(br.nbl[s]):
                    k0 = e0 + b * P
                    n = min(P, e1 - k0)
                    if n <= 0:
                        break
                    col = int(cbase[s]) + b
                    gidx[:n, col] = src_rows[k0 : k0 + n]
                    dstloc[:n, col] = (br.dst[k0 : k0 + n] - w * P).astype(np.float32)
                n0 = w * P
                nn = min(P, br.N - n0)
                if nn > 0:
                    gb = br.batch[n0 : n0 + nn]
                    ind[np.arange(nn), s * G + gb] = 1.0
                    xt[:, s * P : s * P + nn] = xin[n0 : n0 + nn].T
            per_core[c][f"{tag}_gidx"] = gidx
            per_core[c][f"{tag}_dstloc"] = dstloc
            per_core[c][f"{tag}_ind"] = ind
            per_core[c][f"{tag}_xinT"] = xt

    # weights (same on all cores)
    for tag, wname, aname, dname, bname, ename, ebname in (
        ("p", "pg_W", "pg_as", "pg_ad", "pg_b", "pe_w", "pe_b"),
        ("l", "lg_W", "lg_as", "lg_ad", "lg_b", "le_w", "le_b"),
    ):
        Wt = np.stack([_np32(inputs[wname][i]).T for i in range(DEPTH)])
        shared[f"{tag}_WtT"] = np.ascontiguousarray(Wt)  # [4,128,128] lhsT
        asad = np.stack(
            [
                np.concatenate(
                    [_blockdiag(_np32(inputs[aname][i])), _blockdiag(_np32(inputs[dname][i]))],
                    axis=1,
                )
                for i in range(DEPTH)
            ]
        )  # [4, 128, 8]
        shared[f"{tag}_AsAd"] = asad.astype(ml_dtypes.bfloat16)
        shared[f"{tag}_bias"] = np.ascontiguousarray(_np32(inputs[dname.replace("_ad", "_b")]))[
            :, :, None
        ]  # [4,128,1]
        shared[f"{tag}_encT"] = np.ascontiguousarray(_np32(inputs[ename]).T)  # [16,128]
        shared[f"{tag}_bias0"] = (_np32(inputs[ebname]) + te)[:, None]  # [128,1]

    # pooling: reciprocal counts tile [128, 64]
    rc = np.zeros((P, 2 * G), np.float32)
    for tag, br, off in (("p", pb, 0), ("l", lb, G)):
        cnt = np.bincount(br.batch, minlength=G).astype(np.float32)
        rc[:, off : off + G] = 1.0 / np.maximum(cnt, 1.0)
    shared["recip_cnt"] = rc

    # prediction MLP
    W1, W2, w3 = _np32(inputs["pr_w1"]), _np32(inputs["pr_w2"]), _np32(inputs["pr_w3"])
    W1T, W2T = W1.T, W2.T  # [256,256],[256,128]
    shared["mlpW"] = np.ascontiguousarray(
        np.stack(
            [
                W1T[0:128, 0:128], W1T[128:256, 0:128],
                W1T[0:128, 128:256], W1T[128:256, 128:256],
                W2T[0:128, :], W2T[128:256, :],
            ]
        )
    )  # [6,128,128]
    b1 = _np32(inputs["pr_b1"])
    shared["mlpb"] = np.ascontiguousarray(
        np.stack([b1[0:128, None], b1[128:256, None], _np32(inputs["pr_b2"])[:, None]])
    )  # [3,128,1]
    shared["w3col"] = np.ascontiguousarray(w3.reshape(HID, 1))
    meta["b3"] = float(_np32(inputs["pr_b3"])[0])

    # constants: iota row/col, identities
    shared["iota_row"] = np.broadcast_to(np.arange(P, dtype=np.float32)[None, :], (P, P)).copy()
    shared["iota_col"] = np.arange(P, dtype=np.float32)[:, None].copy()
    shared["ident"] = np.eye(P, dtype=np.float32)
    shared["ident_bf"] = np.eye(P, dtype=np.float32).astype(ml_dtypes.bfloat16)

    in_maps = []
    for c in range(N_CORES):
        m = dict(shared)
        m.update(per_core[c])
        in_maps.append(m)
    return meta, in_maps


# ----------------------------------------------------------------------------
# device program
# ----------------------------------------------------------------------------

def _build(meta):
    import concourse.bass as bass
    import concourse.bacc as bacc
    import concourse.mybir as mybir
    import concourse.tile as tile

    f32 = mybir.dt.float32
    bf16 = mybir.dt.bfloat16
    i32 = mybir.dt.int32
    AF = mybir.ActivationFunctionType
    OP = mybir.AluOpType

    p_wc, l_wc = meta["p_wc"], meta["l_wc"]
    p_nbl, l_nbl = meta["p_nbl"], meta["l_nbl"]
    p_btot, l_btot = sum(p_nbl), sum(l_nbl)

    nc = bacc.Bacc("TRN2", target_bir_lowering=False, debug=False, num_devices=N_CORES)

    def din(name, shape, dt=f32):
        return nc.dram_tensor(name, shape, dt, kind="ExternalInput")

    # inputs
    t_in = {}
    for tag, wc, btot in (("p", p_wc, p_btot), ("l", l_wc, l_btot)):
        t_in[f"{tag}_gidx"] = din(f"{tag}_gidx", [P, btot], i32)
        t_in[f"{tag}_dstloc"] = din(f"{tag}_dstloc", [P, btot])
        t_in[f"{tag}_ind"] = din(f"{tag}_ind", [P, wc * G])
        t_in[f"{tag}_xinT"] = din(f"{tag}_xinT", [16, wc * P])
        t_in[f"{tag}_WtT"] = din(f"{tag}_WtT", [DEPTH, P, P])
        t_in[f"{tag}_AsAd"] = din(f"{tag}_AsAd", [DEPTH, P, 8], bf16)
        t_in[f"{tag}_bias"] = din(f"{tag}_bias", [DEPTH, P, 1])
        t_in[f"{tag}_encT"] = din(f"{tag}_encT", [16, P])
        t_in[f"{tag}_bias0"] = din(f"{tag}_bias0", [P, 1])
    for n, shape in (
        ("recip_cnt", [P, 2 * G]), ("mlpW", [6, P, P]), ("mlpb", [3, P, 1]),
        ("w3col", [P, 1]), ("iota_row", [P, P]), ("iota_col", [P, 1]),
        ("ident", [P, P]),
    ):
        t_in[n] = din(n, shape)
    t_in["ident_bf"] = din("ident_bf", [P, P], bf16)

    table_shard = {
        "p": nc.dram_tensor("table_shard_p", [p_wc * P, ROW], bf16),
        "l": nc.dram_tensor("table_shard_l", [l_wc * P, ROW], bf16),
    }
    table_full = {
        "p": nc.dram_tensor("table_full_p", [N_CORES * p_wc * P, ROW], bf16, addr_space="Shared"),
        "l": nc.dram_tensor("table_full_l", [N_CORES * l_wc * P, ROW], bf16, addr_space="Shared"),
    }
    pool_in = nc.dram_tensor("pool_in", [P, 2 * G], f32)
    pool_out = nc.dram_tensor("pool_out", [P, 2 * G], f32, addr_space="Shared")
    out = nc.dram_tensor("out", [1, G], f32, kind="ExternalOutput")

    with tile.TileContext(nc) as tc:
        from contextlib import ExitStack

        with ExitStack() as ctx:
            cons = ctx.enter_context(tc.tile_pool(name="cons", bufs=1))
            sb = ctx.enter_context(tc.tile_pool(name="sb", bufs=1))
            gp = ctx.enter_context(tc.tile_pool(name="gp", bufs=20))
            mp = ctx.enter_context(tc.tile_pool(name="mp", bufs=6))
            uep = ctx.enter_context(tc.tile_pool(name="uep", bufs=6))
            zp = ctx.enter_context(tc.tile_pool(name="zp", bufs=6))
            hwp = ctx.enter_context(tc.tile_pool(name="hwp", bufs=2))
            rowp = ctx.enter_context(tc.tile_pool(name="rowp", bufs=2))
            psA = ctx.enter_context(tc.tile_pool(name="psA", bufs=1, space="PSUM"))
            psB = ctx.enter_context(tc.tile_pool(name="psB", bufs=1, space="PSUM"))
            psC = ctx.enter_context(tc.tile_pool(name="psC", bufs=1, space="PSUM"))
            psAcc = ctx.enter_context(tc.tile_pool(name="psAcc", bufs=2, space="PSUM"))
            psPool = ctx.enter_context(tc.tile_pool(name="psPool", bufs=1, space="PSUM"))

            def load(name, dt=None):
                src = t_in[name]
                shape = list(src.shape)
                if len(shape) == 3:
                    # stacked weights [S, P, F] -> SBUF [P, S*F], slice i at
                    # cols [i*F, (i+1)*F)
                    S, Pp, F = shape
                    t = cons.tile([Pp, S * F], dt or src.dtype, tag=name, name=f"c_{name}")
                    for i in range(S):
                        nc.sync.dma_start(out=t[:, i * F : (i + 1) * F], in_=src[i])
                    t = t.rearrange("p (s f) -> p s f", s=S)
                    return t
                t = cons.tile(shape, dt or src.dtype, tag=name, name=f"c_{name}")
                nc.sync.dma_start(out=t[:], in_=src[:])
                return t

            iota_row = load("iota_row")
            iota_col = load("iota_col")
            ident = load("ident")
            ident_bf = load("ident_bf")
            recip_cnt = load("recip_cnt")
            mlpW = load("mlpW")
            mlpb = load("mlpb")
            w3col = load("w3col")
            consts = {}
            for tag in ("p", "l"):
                for n in ("gidx", "dstloc", "ind", "xinT", "WtT", "AsAd", "bias", "encT", "bias0"):
                    consts[f"{tag}_{n}"] = load(f"{tag}_{n}")

            # h_T state per branch [128, wc*128] f32 (feature-major)
            hT = {
                "p": sb.tile([P, p_wc * P], f32, tag="p_hT", name="p_hT"),
                "l": sb.tile([P, l_wc * P], f32, tag="l_hT", name="l_hT"),
            }
            # pooled psums (layer 3)
            pool_ps = {}

            # ---------------- encoder: h0_T = enc(x) + te ---------------
            for tag, wc in (("p", p_wc), ("l", l_wc)):
                ncols = wc * P
                nch = -(-ncols // 480)
                for cix in range(nch):
                    c0 = cix * 480
                    cw = min(480, ncols - c0)
                    ps = psA.tile([P, 480], f32, space="PSUM", tag="a", name="encps")
                    nc.tensor.matmul(
                        ps[:, :cw],
                        lhsT=consts[f"{tag}_encT"][:, :],
                        rhs=consts[f"{tag}_xinT"][:, c0 : c0 + cw],
                        start=True, stop=True,
                    )
                    nc.vector.tensor_scalar_add(
                        hT[tag][:, c0 : c0 + cw], ps[:, :cw], consts[f"{tag}_bias0"][:, 0:1]
                    )

            # ---------------- per-layer (pipelined per branch) ----------------
            def build_table(tag, layer):
                wc = {"p": p_wc, "l": l_wc}[tag]
                xT = sb.tile([P, wc * P], bf16, tag=f"{tag}_xT", name=f"{tag}_xT_{layer}")
                for s in range(wc):
                    c0 = s * P
                    ps = psA.tile([P, P], f32, space="PSUM", tag="a", name="bldps")
                    nc.tensor.matmul(
                        ps[:, :],
                        lhsT=consts[f"{tag}_WtT"][:, layer],
                        rhs=hT[tag][:, c0 : c0 + P],
                        start=True, stop=True,
                    )
                    nc.vector.tensor_copy(out=xT[:, c0 : c0 + P], in_=ps[:, :])
                    xstage = hwp.tile([P, P], f32, tag="xstage")
                    nc.vector.tensor_copy(out=xstage[:], in_=ps[:, :])
                    aps = psC.tile([8, P], f32, space="PSUM", tag="c", name="aps")
                    nc.tensor.matmul(
                        aps[:, :],
                        lhsT=consts[f"{tag}_AsAd"][:, layer],
                        rhs=xT[:, c0 : c0 + P],
                        start=True, stop=True,
                    )
                    st8 = zp.tile([8, P], f32, tag="st8")
                    nc.vector.tensor_copy(out=st8[:], in_=aps[:, :])
                    row = rowp.tile([P, ROW], bf16, tag="row")
                    xps = psB.tile([P, P], f32, space="PSUM", tag="b", name="xps")
                    nc.tensor.transpose(xps[:, :], xstage[:], ident[:, :])
                    nc.vector.tensor_copy(out=row[:, 0:HID], in_=xps[:, :])
                    aps2 = psB.tile([P, P], f32, space="PSUM", tag="b", name="aps2")
                    nc.tensor.transpose(aps2[:, 0:8], st8[:, :], ident[0:8, 0:8])
                    nc.vector.tensor_copy(out=row[:, HID : HID + 8], in_=aps2[:, 0:8])
                    nc.sync.dma_start(out=table_shard[tag][c0 : c0 + P, :], in_=row[:])
                nc.gpsimd.collective_compute(
                    "AllGather", mybir.AluOpType.bypass,
                    replica_groups=[list(range(N_CORES))],
                    ins=[table_shard[tag][:, :]], outs=[table_full[tag][:, :]],
                )

            def edge_phase(tag, layer):
                wc = {"p": p_wc, "l": l_wc}[tag]
                nbl = {"p": p_nbl, "l": l_nbl}[tag]
                last = layer == DEPTH - 1
                gidx = consts[f"{tag}_gidx"]
                dstloc = consts[f"{tag}_dstloc"]
                if last:
                    pool_ps[tag] = psPool.tile(
                        [P, G], f32, space="PSUM", tag=f"pp{tag}", name=f"pp{tag}"
                    )
                cbase = 0
                for s in range(wc):
                    acc = psAcc.tile([P, 132], f32, space="PSUM", tag="acc")
                    # --- self-loop block: contiguous rows of our own shard ---
                    srow = zp.tile([P, 8], bf16, tag="srow")
                    nc.sync.dma_start(
                        out=srow[:],
                        in_=table_shard[tag][s * P : (s + 1) * P, HID : HID + 8],
                    )
                    xwin = gp.tile([P, HID], bf16, tag="xwin")
                    nc.sync.dma_start(
                        out=xwin[:], in_=table_shard[tag][s * P : (s + 1) * P, 0:HID]
                    )
                    z = zp.tile([P, 4], f32, tag="z")
                    nc.vector.tensor_add(out=z[:], in0=srow[:, 0:4], in1=srow[:, 4:8])
                    z2 = zp.tile([P, 4], f32, tag="z2")
                    nc.vector.tensor_scalar_mul(z2[:], z[:], 0.2)
                    nc.vector.tensor_tensor(out=z[:], in0=z[:], in1=z2[:], op=OP.max)
                    ex = zp.tile([P, 4], f32, tag="ex")
                    nc.scalar.activation(ex[:], z[:], AF.Exp)
                    ue = uep.tile([P, 132], bf16, tag="ue")
                    nc.vector.tensor_copy(out=ue[:, HID : HID + 4], in_=ex[:])
                    nc.vector.tensor_tensor(
                        out=ue[:, 0:HID], in0=xwin[:],
                        in1=ue[:, HID : HID + 4].broadcast_to([P, 4, 32]),
                        op=OP.mult,
                    )
                    nc.tensor.matmul(
                        acc[:, :], lhsT=ident_bf[:, :], rhs=ue[:],
                        start=True, stop=(nbl[s] == 0),
                    )
                    # --- gathered edge blocks ---
                    adn_win = srow  # cols 4:8 are adn
                    for b in range(nbl[s]):
                        col = cbase + b
                        g = gp.tile([P, ROW], bf16, tag="g")
                        nc.gpsimd.indirect_dma_start(
                            out=g[:], out_offset=None, in_=table_full[tag][:, :],
                            in_offset=bass.IndirectOffsetOnAxis(
                                ap=gidx[:, col : col + 1], axis=0
                            ),
                        )
                        dcol = dstloc[:, col : col + 1]
                        msb = mp.tile([P, P], bf16, tag="msb")
                        nc.vector.tensor_tensor(
                            out=msb[:], in0=iota_row[:, :],
                            in1=dcol.to_broadcast([P, P]), op=OP.is_equal,
                        )
                        bc = psA.tile([P, P], f32, space="PSUM", tag="a", name="bc")
                        nc.tensor.transpose(bc[:, :], dcol.to_broadcast([P, P]), ident[:, :])
                        mt = mp.tile([P, P], bf16, tag="mt")
                        nc.vector.tensor_scalar(
                            out=mt[:], in0=bc[:, :], scalar1=iota_col[:, 0:1],
                            scalar2=None, op0=OP.is_equal,
                        )
                        adn_ps = psC.tile([P, 4], f32, space="PSUM", tag="c", name="adnps")
                        nc.tensor.matmul(
                            adn_ps[:, :], lhsT=mt[:], rhs=adn_win[:, 4:8],
                            start=True, stop=True,
                        )
                        z = zp.tile([P, 4], f32, tag="z")
                        nc.vector.tensor_copy(out=z[:], in_=g[:, HID : HID + 4])
                        nc.vector.tensor_add(out=z[:], in0=z[:], in1=adn_ps[:, :])
                        z2 = zp.tile([P, 4], f32, tag="z2")
                        nc.vector.tensor_scalar_mul(z2[:], z[:], 0.2)
                        nc.vector.tensor_tensor(out=z[:], in0=z[:], in1=z2[:], op=OP.max)
                        ex = zp.tile([P, 4], f32, tag="ex")
                        nc.scalar.activation(ex[:], z[:], AF.Exp)
                        ue = uep.tile([P, 132], bf16, tag="ue")
                        nc.vector.tensor_copy(out=ue[:, HID : HID + 4], in_=ex[:])
                        nc.vector.tensor_tensor(
                            out=ue[:, 0:HID], in0=g[:, 0:HID],
                            in1=ue[:, HID : HID + 4].broadcast_to([P, 4, 32]),
                            op=OP.mult,
                        )
                        nc.tensor.matmul(
                            acc[:, :], lhsT=msb[:], rhs=ue[:],
                            start=False, stop=(b == nbl[s] - 1),
                        )
                    cbase += nbl[s]
                    # --- window epilogue ---
                    ssb = zp.tile([P, 4], f32, tag="ssb")
                    nc.vector.tensor_scalar_add(ssb[:], acc[:, HID : HID + 4], 1e-16)
                    rsb = zp.tile([P, 4], f32, tag="rsb")
                    nc.vector.reciprocal(rsb[:], ssb[:])
                    hw = hwp.tile([P, P], f32, tag="hw")
                    nc.vector.tensor_tensor(
                        out=hw[:], in0=acc[:, 0:HID],
                        in1=rsb[:].broadcast_to([P, 4, 32]), op=OP.mult,
                    )
                    nc.vector.tensor_scalar(
                        out=hw[:], in0=hw[:], scalar1=consts[f"{tag}_bias"][:, layer],
                        scalar2=0.0, op0=OP.add, op1=OP.max,
                    )
                    if last:
                        nc.tensor.matmul(
                            pool_ps[tag][:, :], lhsT=hw[:],
                            rhs=consts[f"{tag}_ind"][:, s * G : (s + 1) * G],
                            start=(s == 0), stop=(s == wc - 1),
                        )
                    else:
                        htp = psB.tile([P, P], f32, space="PSUM", tag="b", name="htp")
                        nc.tensor.transpose(htp[:, :], hw[:], ident[:, :])
                        nc.vector.tensor_copy(
                            out=hT[tag][:, s * P : (s + 1) * P], in_=htp[:, :]
                        )

            # pipeline: AG(tag, i) overlaps the other branch's edge phase
            build_table("p", 0)
            build_table("l", 0)
            for layer in range(DEPTH):
                edge_phase("p", layer)
                if layer < DEPTH - 1:
                    build_table("p", layer + 1)
                edge_phase("l", layer)
                if layer < DEPTH - 1:
                    build_table("l", layer + 1)

            pool_sb = hwp.tile([P, 2 * G], f32, tag="poolsb")
            nc.vector.tensor_copy(out=pool_sb[:, 0:G], in_=pool_ps["p"][:, :])
            nc.vector.tensor_copy(out=pool_sb[:, G : 2 * G], in_=pool_ps["l"][:, :])
            nc.sync.dma_start(out=pool_in[:, :], in_=pool_sb[:])
            nc.gpsimd.collective_compute(
                "AllReduce", mybir.AluOpType.add,
                replica_groups=[list(range(N_CORES))],
                ins=[pool_in[:, :]], outs=[pool_out[:, :]],
            )
            jt = hwp.tile([P, 2 * G], f32, tag="jt")
            nc.sync.dma_start(out=jt[:], in_=pool_out[:, :])
            nc.vector.tensor_tensor(out=jt[:], in0=jt[:], in1=recip_cnt[:, :], op=OP.mult)

            # h1 = relu(W1 @ j + b1): two 128-halves
            h1 = []
            for half in range(2):
                ps = psPool.tile([P, G], f32, space="PSUM", tag=f"pp{'p' if half else 'l'}", name=f"mlp{half}")
                nc.tensor.matmul(ps[:, :], lhsT=mlpW[:, 2 * half], rhs=jt[:, 0:G],
                                 start=True, stop=False)
                nc.tensor.matmul(ps[:, :], lhsT=mlpW[:, 2 * half + 1], rhs=jt[:, G : 2 * G],
                                 start=False, stop=True)
                t = zp.tile([P, G], f32, tag=f"h1{half}")
                nc.vector.tensor_scalar(
                    out=t[:], in0=ps[:, :], scalar1=mlpb[:, half],
                    scalar2=0.0, op0=OP.add, op1=OP.max,
                )
                h1.append(t)
            ps = psPool.tile([P, G], f32, space="PSUM", tag="ppp", name="mlp2")
            nc.tensor.matmul(ps[:, :], lhsT=mlpW[:, 4], rhs=h1[0][:], start=True, stop=False)
            nc.tensor.matmul(ps[:, :], lhsT=mlpW[:, 5], rhs=h1[1][:], start=False, stop=True)
            h2 = zp.tile([P, G], f32, tag="h2")
            nc.vector.tensor_scalar(
                out=h2[:], in0=ps[:, :], scalar1=mlpb[:, 2],
                scalar2=0.0, op0=OP.add, op1=OP.max,
            )
            pps = psC.tile([1, G], f32, space="PSUM", tag="c", name="predps")
            nc.tensor.matmul(pps[:, :], lhsT=w3col[:], rhs=h2[:], start=True, stop=True)
            pred = zp.tile([1, G], f32, tag="predsb")
            nc.vector.tensor_scalar_add(pred[:], pps[:, :], meta["b3"])
            nc.sync.dma_start(out=out[0:1, :], in_=pred[:])

    nc.compile()
    return nc


# ----------------------------------------------------------------------------
# entry point
# ----------------------------------------------------------------------------

LAST_EXEC_NS = None
LAST_RESULT = None


def kernel(_trace=False, **inputs) -> np.ndarray:
    global LAST_EXEC_NS, LAST_RESULT
    from concourse.bass_utils import run_bass_kernel_spmd

    meta, in_maps = _prep(inputs)
    nc = _build(meta)
    res = run_bass_kernel_spmd(nc, in_maps, list(range(N_CORES)), trace=_trace)
    LAST_EXEC_NS = res.exec_time_ns
    LAST_RESULT = res
    return np.asarray(res.results[0]["out"], np.float32).reshape(G)



# revision 9
# speedup vs baseline: 1.0150x; 1.0150x over previous
"""Trainium2 Bass kernel for DiffDock-style GAT model (8 NeuronCores, SPMD).

Strategy (v2 — instruction-count-oriented rewrite):
- Edges (+self-loops) sorted by destination; destinations tiled into 128-node
  windows; windows assigned snake-wise to the 8 cores. Node table rows
  [x (128) | asn (4) | adn (4)] bf16 are laid out rank-major so one AllGather
  per layer/branch publishes every core's shard.
- Edge processing is batched into multi-window "chunks" (<=3 windows,
  <=56 edge blocks). Per chunk: ONE indirect DMA gathers all src rows
  (132 elems each), ONE indirect DMA gathers the 4 adn[dst] elems per edge
  (element_offset into the same table), then the whole per-edge softmax
  numerator pipeline (z = asn+adn, leaky-relu, exp, x*ex) runs as ~6 chunk-wide
  strided vector/scalar instructions. Scatter-to-destination is one
  PSUM-accumulated matmul per 128-edge block with a DVE-built indicator.
- Softmax uses no max-subtraction (|e| < 2 for these inputs), so alpha
  normalization commutes with the scatter and is applied per window.
- Self-loops use table rows kept resident in SBUF; their exp/ue terms are
  computed once per layer for all windows.
- Mean-pool via batch-indicator matmuls, AllReduce of [128,64] pooled
  features, then the small prediction MLP replicated on every core.
"""

import numpy as np
import ml_dtypes

HID = 128
HEADS = 4
DEPTH = 4
TDIM = 32
G = 32
P = 128
N_CORES = 8
ROW = 136   # SBUF table row: x(128) + asn(4) + adn(4)
ROWP = 256  # padded DRAM table row (dma_gather needs 256B-multiple elem_size)
GROW = 132  # per-edge matmul rhs: x(128) + ex(4)
KMAX = 36   # max edge blocks per chunk
WMAX = 3    # max windows per chunk (PSUM bank: 3*132*4B < 2KB)


# ----------------------------------------------------------------------------
# host-side helpers
# ----------------------------------------------------------------------------

def _np32(a):
    return np.asarray(a, dtype=np.float32)


def _time_embedding(t_w1, t_b1, t_w2, t_b2):
    # sinusoidal_emb(0) = [sin(0)*16, cos(0)*16] = [0]*16 + [1]*16
    te = np.concatenate([np.zeros(16, np.float32), np.ones(16, np.float32)])
    h = te @ _np32(t_w1).T + _np32(t_b1)
    h = h * (1.0 / (1.0 + np.exp(-h)))  # silu
    return h @ _np32(t_w2).T + _np32(t_b2)  # [128]


def _blockdiag(a):
    # a: [HEADS, HID//HEADS] -> [HID, HEADS] block diagonal
    m = np.zeros((HID, HEADS), np.float32)
    for h in range(HEADS):
        m[32 * h : 32 * (h + 1), h] = a[h]
    return m


class _Branch:
    """Host-side graph prep for one branch (protein or ligand).

    Self-loops are NOT added to the edge list; they are handled as a cheap
    contiguous "self block" per window (identity indicator, no gather).
    Windows are dealt snake-wise across cores by descending block count so
    per-slot maxima stay tight and cores stay balanced.
    """

    def __init__(self, x, edge_index, batch):
        N = x.shape[0]
        src = np.asarray(edge_index[0], np.int64)
        dst = np.asarray(edge_index[1], np.int64)
        order = np.argsort(dst, kind="stable")
        self.src = src[order]
        self.dst = dst[order]
        self.N = N
        self.batch = np.asarray(batch, np.int64)
        n_win = -(-N // P)
        n_win = -(-n_win // N_CORES) * N_CORES  # multiple of 8
        self.n_win = n_win
        self.wc = n_win // N_CORES  # windows per core
        win = self.dst // P
        wcnt = np.bincount(win, minlength=n_win)
        self.wcnt = wcnt
        self.wstart = np.concatenate([[0], np.cumsum(wcnt)])
        # snake-deal windows (desc by block count) across cores
        blocks = -(-wcnt // P)
        rank = np.argsort(-blocks, kind="stable")
        self.win_of = np.zeros((N_CORES, self.wc), np.int64)  # [core, slot] -> w
        core_of = np.zeros(n_win, np.int64)
        slot_of = np.zeros(n_win, np.int64)
        for s in range(self.wc):
            grp = rank[s * N_CORES : (s + 1) * N_CORES]
            cores = range(N_CORES) if s % 2 == 0 else range(N_CORES - 1, -1, -1)
            for c, w in zip(cores, grp):
                self.win_of[c, s] = w
                core_of[w] = c
                slot_of[w] = s
        self.core_of, self.slot_of = core_of, slot_of
        # per-slot block count = max over cores (shared SPMD program shape)
        self.nbl = [
            int(max(blocks[self.win_of[c, s]] for c in range(N_CORES)))
            for s in range(self.wc)
        ]
        # chunking: consecutive slots, <=WMAX windows and <=KMAX blocks each
        self.chunks = []  # list of (s0, nw, kblocks, cbase)
        s0, k, cb = 0, 0, 0
        for s in range(self.wc):
            if s > s0 and (s - s0 >= WMAX or k + self.nbl[s] > KMAX):
                self.chunks.append((s0, s - s0, k, cb))
                cb += k
                s0, k = s, 0
            k += self.nbl[s]
        self.chunks.append((s0, self.wc - s0, k, cb))
        assert all(c[2] <= KMAX and c[1] <= WMAX for c in self.chunks)

    def shard_slot(self, node):
        """(core, row_within_branch_shard) for each node id."""
        w = node // P
        return self.core_of[w], self.slot_of[w] * P + node % P


def _prep(inputs):
    pb = _Branch(inputs["protein_x"], inputs["protein_edge_index"], inputs["protein_batch"])
    lb = _Branch(inputs["ligand_x"], inputs["ligand_edge_index"], inputs["ligand_batch"])
    meta = {
        "p_wc": pb.wc, "l_wc": lb.wc,
        "p_nbl": pb.nbl, "l_nbl": lb.nbl,
        "p_chunks": pb.chunks, "l_chunks": lb.chunks,
    }

    def table_row(branch, node):
        # branch tables are separate; rows are rank-major
        c, r = branch.shard_slot(node)
        return c * branch.wc * P + r

    te = _time_embedding(inputs["t_w1"], inputs["t_b1"], inputs["t_w2"], inputs["t_b2"])

    per_core = [dict() for _ in range(N_CORES)]
    shared = {}

    for tag, br in (("p", pb), ("l", lb)):
        wc = br.wc
        btot = sum(br.nbl)
        cbase = np.concatenate([[0], np.cumsum(br.nbl)]).astype(np.int64)
        # global row index of each edge's src in the full table
        src_rows = table_row(br, br.src)
        xin = _np32(inputs[f"{'protein' if tag == 'p' else 'ligand'}_x"])
        for c in range(N_CORES):
            gpk = np.zeros((P, btot * 8), np.int16)
            dstloc = np.full((P, btot), -1.0, np.float32)
            dstlocT = np.full((1, btot * P), 0.0, np.float32)
            ind = np.zeros((P, wc * G), np.float32)
            xt = np.zeros((16, wc * P), np.float32)
            for s in range(wc):
                w = int(br.win_of[c, s])
                e0, e1 = br.wstart[w], br.wstart[w + 1]
                for b in range(br.nbl[s]):
                    k0 = e0 + b * P
                    n = min(P, e1 - k0)
                    if n <= 0:
                        break
                    col = int(cbase[s]) + b
                    jj = np.arange(n)
                    gpk[jj % 16, col * 8 + jj // 16] = src_rows[k0 : k0 + n].astype(np.int16)
                    dl = (br.dst[k0 : k0 + n] - w * P).astype(np.int64)
                    dstloc[:n, col] = dl.astype(np.float32)
                    dstlocT[0, col * P : col * P + n] = dl.astype(np.float32)
                n0 = w * P
                nn = min(P, br.N - n0)
                if nn > 0:
                    gb = br.batch[n0 : n0 + nn]
                    ind[np.arange(nn), s * G + gb] = 1.0
                    xt[:, s * P : s * P + nn] = xin[n0 : n0 + nn].T
            for cc in range(1, 8):
                gpk[16 * cc : 16 * (cc + 1)] = gpk[0:16]
            per_core[c][f"{tag}_gpk"] = gpk
            per_core[c][f"{tag}_dstloc"] = dstloc.astype(ml_dtypes.bfloat16)
            per_core[c][f"{tag}_dstlocT"] = dstlocT.astype(ml_dtypes.bfloat16)
            per_core[c][f"{tag}_ind"] = ind
            per_core[c][f"{tag}_xinT"] = xt

    # weights (same on all cores)
    for tag, wname, aname, dname, bname, ename, ebname in (
        ("p", "pg_W", "pg_as", "pg_ad", "pg_b", "pe_w", "pe_b"),
        ("l", "lg_W", "lg_as", "lg_ad", "lg_b", "le_w", "le_b"),
    ):
        Wt = np.stack([_np32(inputs[wname][i]).T for i in range(DEPTH)])
        shared[f"{tag}_WtT"] = np.ascontiguousarray(Wt).astype(ml_dtypes.bfloat16)  # [4,128,128] lhsT
        asad = np.stack(
            [
                np.concatenate(
                    [_blockdiag(_np32(inputs[aname][i])), _blockdiag(_np32(inputs[dname][i]))],
                    axis=1,
                )
                for i in range(DEPTH)
            ]
        )  # [4, 128, 8]
        shared[f"{tag}_AsAd"] = asad.astype(ml_dtypes.bfloat16)
        shared[f"{tag}_bias"] = np.ascontiguousarray(_np32(inputs[dname.replace("_ad", "_b")]))[
            :, :, None
        ]  # [4,128,1]
        shared[f"{tag}_encT"] = np.ascontiguousarray(_np32(inputs[ename]).T)  # [16,128]
        shared[f"{tag}_bias0"] = (_np32(inputs[ebname]) + te)[:, None]  # [128,1]

    # pooling: reciprocal counts tile [128, 64]
    rc = np.zeros((P, 2 * G), np.float32)
    for tag, br, off in (("p", pb, 0), ("l", lb, G)):
        cnt = np.bincount(br.batch, minlength=G).astype(np.float32)
        rc[:, off : off + G] = 1.0 / np.maximum(cnt, 1.0)
    shared["recip_cnt"] = rc

    # prediction MLP
    W1, W2, w3 = _np32(inputs["pr_w1"]), _np32(inputs["pr_w2"]), _np32(inputs["pr_w3"])
    W1T, W2T = W1.T, W2.T  # [256,256],[256,128]
    shared["mlpW"] = np.ascontiguousarray(
        np.stack(
            [
                W1T[0:128, 0:128], W1T[128:256, 0:128],
                W1T[0:128, 128:256], W1T[128:256, 128:256],
                W2T[0:128, :], W2T[128:256, :],
            ]
        )
    )  # [6,128,128]
    b1 = _np32(inputs["pr_b1"])
    shared["mlpb"] = np.ascontiguousarray(
        np.stack([b1[0:128, None], b1[128:256, None], _np32(inputs["pr_b2"])[:, None]])
    )  # [3,128,1]
    shared["w3col"] = np.ascontiguousarray(w3.reshape(HID, 1))
    meta["b3"] = float(_np32(inputs["pr_b3"])[0])

    # constants
    shared["iota_tiled"] = np.broadcast_to(
        np.tile(np.arange(P, dtype=np.float32), KMAX)[None, :], (P, KMAX * P)
    ).astype(ml_dtypes.bfloat16).copy()
    shared["ident"] = np.eye(P, dtype=np.float32)
    shared["iota_col"] = np.arange(P, dtype=np.float32)[:, None].copy()
    shared["ones1"] = np.ones((1, P), np.float32).astype(ml_dtypes.bfloat16)
    shared["ident_bf"] = np.eye(P, dtype=np.float32).astype(ml_dtypes.bfloat16)

    in_maps = []
    for c in range(N_CORES):
        m = dict(shared)
        m.update(per_core[c])
        in_maps.append(m)
    return meta, in_maps


# ----------------------------------------------------------------------------
# device program
# ----------------------------------------------------------------------------

def _build(meta):
    import concourse.bass as bass
    import concourse.bacc as bacc
    import concourse.mybir as mybir
    import concourse.tile as tile

    f32 = mybir.dt.float32
    bf16 = mybir.dt.bfloat16
    i32 = mybir.dt.int32
    i16 = mybir.dt.int16
    AF = mybir.ActivationFunctionType
    OP = mybir.AluOpType

    p_wc, l_wc = meta["p_wc"], meta["l_wc"]
    p_nbl, l_nbl = meta["p_nbl"], meta["l_nbl"]
    p_btot, l_btot = sum(p_nbl), sum(l_nbl)
    chunks = {"p": meta["p_chunks"], "l": meta["l_chunks"]}
    nbl_of = {"p": p_nbl, "l": l_nbl}
    wc_of = {"p": p_wc, "l": l_wc}

    nc = bacc.Bacc("TRN2", target_bir_lowering=False, debug=False, num_devices=N_CORES)

    def din(name, shape, dt=f32):
        return nc.dram_tensor(name, shape, dt, kind="ExternalInput")

    # inputs
    t_in = {}
    for tag, wc, btot in (("p", p_wc, p_btot), ("l", l_wc, l_btot)):
        t_in[f"{tag}_gpk"] = din(f"{tag}_gpk", [P, btot * 8], i16)
        t_in[f"{tag}_dstloc"] = din(f"{tag}_dstloc", [P, btot], bf16)
        t_in[f"{tag}_dstlocT"] = din(f"{tag}_dstlocT", [1, btot * P], bf16)
        t_in[f"{tag}_ind"] = din(f"{tag}_ind", [P, wc * G])
        t_in[f"{tag}_xinT"] = din(f"{tag}_xinT", [16, wc * P])
        t_in[f"{tag}_WtT"] = din(f"{tag}_WtT", [DEPTH, P, P], bf16)
        t_in[f"{tag}_AsAd"] = din(f"{tag}_AsAd", [DEPTH, P, 8], bf16)
        t_in[f"{tag}_bias"] = din(f"{tag}_bias", [DEPTH, P, 1])
        t_in[f"{tag}_encT"] = din(f"{tag}_encT", [16, P])
        t_in[f"{tag}_bias0"] = din(f"{tag}_bias0", [P, 1])
    for n, shape in (
        ("recip_cnt", [P, 2 * G]), ("mlpW", [6, P, P]), ("mlpb", [3, P, 1]),
        ("w3col", [P, 1]), ("ident", [P, P]), ("iota_col", [P, 1]),
    ):
        t_in[n] = din(n, shape)
    t_in["iota_tiled"] = din("iota_tiled", [P, KMAX * P], bf16)
    t_in["ones1"] = din("ones1", [1, P], bf16)
    t_in["ident_bf"] = din("ident_bf", [P, P], bf16)

    table_shard = {
        "p": nc.dram_tensor("table_shard_p", [p_wc * P, ROWP], bf16),
        "l": nc.dram_tensor("table_shard_l", [l_wc * P, ROWP], bf16),
    }
    table_full = {
        "p": nc.dram_tensor("table_full_p", [N_CORES * p_wc * P, ROWP], bf16, addr_space="Shared"),
        "l": nc.dram_tensor("table_full_l", [N_CORES * l_wc * P, ROWP], bf16, addr_space="Shared"),
    }
    pool_in = nc.dram_tensor("pool_in", [P, 2 * G], f32)
    pool_out = nc.dram_tensor("pool_out", [P, 2 * G], f32, addr_space="Shared")
    out = nc.dram_tensor("out", [1, G], f32, kind="ExternalOutput")

    with tile.TileContext(nc) as tc:
        from contextlib import ExitStack

        with ExitStack() as ctx:
            cons = ctx.enter_context(tc.tile_pool(name="cons", bufs=1))
            sb = ctx.enter_context(tc.tile_pool(name="sb", bufs=1))
            gp = ctx.enter_context(tc.tile_pool(name="gp", bufs=2))
            ap_ = ctx.enter_context(tc.tile_pool(name="ap", bufs=2))
            mp = ctx.enter_context(tc.tile_pool(name="mp", bufs=2))
            zp = ctx.enter_context(tc.tile_pool(name="zp", bufs=3))
            hwp = ctx.enter_context(tc.tile_pool(name="hwp", bufs=2))
            uesp = ctx.enter_context(tc.tile_pool(name="uesp", bufs=2))
            psA = ctx.enter_context(tc.tile_pool(name="psA", bufs=1, space="PSUM"))
            psB = ctx.enter_context(tc.tile_pool(name="psB", bufs=1, space="PSUM"))
            psC = ctx.enter_context(tc.tile_pool(name="psC", bufs=1, space="PSUM"))
            psAcc = ctx.enter_context(tc.tile_pool(name="psAcc", bufs=2, space="PSUM"))
            psPool = ctx.enter_context(tc.tile_pool(name="psPool", bufs=1, space="PSUM"))
            psD = ctx.enter_context(tc.tile_pool(name="psD", bufs=1, space="PSUM"))
            mtp = ctx.enter_context(tc.tile_pool(name="mtp", bufs=1))
            dtp = ctx.enter_context(tc.tile_pool(name="dtp", bufs=2))

            def load(name, dt=None):
                src = t_in[name]
                shape = list(src.shape)
                if len(shape) == 3:
                    # stacked weights [S, P, F] -> SBUF [P, S*F], slice i at
                    # cols [i*F, (i+1)*F)
                    S, Pp, F = shape
                    t = cons.tile([Pp, S * F], dt or src.dtype, tag=name, name=f"c_{name}")
                    for i in range(S):
                        nc.sync.dma_start(out=t[:, i * F : (i + 1) * F], in_=src[i])
                    t = t.rearrange("p (s f) -> p s f", s=S)
                    return t
                t = cons.tile(shape, dt or src.dtype, tag=name, name=f"c_{name}")
                nc.sync.dma_start(out=t[:], in_=src[:])
                return t

            iota_tiled = load("iota_tiled")
            iota_col = load("iota_col")
            ones1 = load("ones1")
            ident = load("ident")
            ident_bf = load("ident_bf")
            recip_cnt = load("recip_cnt")
            mlpW = load("mlpW")
            mlpb = load("mlpb")
            w3col = load("w3col")
            consts = {}
            for tag in ("p", "l"):
                for n in ("gpk", "dstloc", "ind", "xinT", "WtT", "AsAd",
                          "bias", "encT", "bias0"):
                    consts[f"{tag}_{n}"] = load(f"{tag}_{n}")

            # persistent per-branch state
            hT = {   # node features, feature-major bf16
                "p": sb.tile([P, p_wc * P], bf16, tag="p_hT", name="p_hT"),
                "l": sb.tile([P, l_wc * P], bf16, tag="l_hT", name="l_hT"),
            }
            rows_sb = {  # node-major table rows for the current layer
                "p": sb.tile([P, p_wc * ROW], bf16, tag="p_rows", name="p_rows"),
                "l": sb.tile([P, l_wc * ROW], bf16, tag="l_rows", name="l_rows"),
            }
            pool_ps = {}

            # ---------------- encoder: h0_T = enc(x) + te ---------------
            for tag in ("p", "l"):
                ncols = wc_of[tag] * P
                nch = -(-ncols // 480)
                for cix in range(nch):
                    c0 = cix * 480
                    cw = min(480, ncols - c0)
                    ps = psA.tile([P, 480], f32, space="PSUM", tag="a", name="encps")
                    nc.tensor.matmul(
                        ps[:, :cw],
                        lhsT=consts[f"{tag}_encT"][:, :],
                        rhs=consts[f"{tag}_xinT"][:, c0 : c0 + cw],
                        start=True, stop=True,
                    )
                    nc.vector.tensor_scalar_add(
                        hT[tag][:, c0 : c0 + cw], ps[:, :cw], consts[f"{tag}_bias0"][:, 0:1]
                    )

            # ---------------- table build + AllGather ----------------
            def build_table(tag, layer):
                wc = wc_of[tag]
                rows = rows_sb[tag]
                for s in range(wc):
                    c0 = s * P
                    ps = psA.tile([P, P], f32, space="PSUM", tag="a", name="bldps")
                    nc.tensor.matmul(
                        ps[:, :],
                        lhsT=consts[f"{tag}_WtT"][:, layer],
                        rhs=hT[tag][:, c0 : c0 + P],
                        start=True, stop=True,
                    )
                    xw = hwp.tile([P, P], bf16, tag="xw")
                    nc.vector.tensor_copy(out=xw[:], in_=ps[:, :])
                    aps = psC.tile([8, P], f32, space="PSUM", tag="c", name="aps")
                    nc.tensor.matmul(
                        aps[:, :],
                        lhsT=consts[f"{tag}_AsAd"][:, layer],
                        rhs=xw[:],
                        start=True, stop=True,
                    )
                    st8 = zp.tile([8, P], bf16, tag="st8")
                    nc.vector.tensor_copy(out=st8[:], in_=aps[:, :])
                    xps = psB.tile([P, P], bf16, space="PSUM", tag="b", name="xps")
                    nc.tensor.transpose(xps[:, :], xw[:], ident_bf[:, :])
                    nc.vector.tensor_copy(out=rows[:, s * ROW : s * ROW + HID], in_=xps[:, :])
                    aps2 = psB.tile([P, P], bf16, space="PSUM", tag="b", name="aps2")
                    nc.tensor.transpose(aps2[:, 0:8], st8[:, :], ident_bf[0:8, 0:8])
                    nc.vector.tensor_copy(
                        out=rows[:, s * ROW + HID : s * ROW + ROW], in_=aps2[:, 0:8]
                    )
                    nc.sync.dma_start(
                        out=table_shard[tag][c0 : c0 + P, 0:ROW],
                        in_=rows[:, s * ROW : (s + 1) * ROW],
                    )
                nc.gpsimd.collective_compute(
                    "AllGather", mybir.AluOpType.bypass,
                    replica_groups=[list(range(N_CORES))],
                    ins=[table_shard[tag][:, :]], outs=[table_full[tag][:, :]],
                )

            # ---------------- edge phase ----------------
            def edge_phase(tag, layer):
                wc = wc_of[tag]
                nbl = nbl_of[tag]
                last = layer == DEPTH - 1
                gpk = consts[f"{tag}_gpk"]
                dstloc = consts[f"{tag}_dstloc"]
                dstlocT_d = t_in[f"{tag}_dstlocT"]
                rows3 = rows_sb[tag].rearrange("p (s r) -> p s r", r=ROW)
                if last:
                    pool_ps[tag] = psPool.tile(
                        [P, G], f32, space="PSUM", tag=f"pp{tag}", name=f"pp{tag}"
                    )

                # self-loop terms for ALL windows of this branch at once:
                # z = asn_self + adn_self ; ex = exp(leaky_relu(z))
                # ues[:, s*132:(s+1)*132] = [x_self * ex | ex]
                zs = zp.tile([P, wc * 4], f32, tag="zs")
                nc.vector.tensor_add(
                    out=zs[:], in0=rows3[:, :, HID : HID + 4], in1=rows3[:, :, HID + 4 : ROW]
                )
                zs2 = zp.tile([P, wc * 4], f32, tag="zs2")
                nc.vector.tensor_scalar_mul(zs2[:], zs[:], 0.2)
                nc.vector.tensor_tensor(out=zs[:], in0=zs[:], in1=zs2[:], op=OP.max)
                exs = zp.tile([P, wc * 4], bf16, tag="exs")
                nc.scalar.activation(exs[:], zs[:], AF.Exp)
                ues = uesp.tile([P, wc * GROW], bf16, tag=f"ues{tag}")
                ues3 = ues.rearrange("p (s r) -> p s r", r=GROW)
                nc.vector.tensor_copy(out=ues3[:, :, HID : GROW], in_=exs[:])
                nc.vector.tensor_tensor(
                    out=ues3[:, :, 0:HID], in0=rows3[:, :, 0:HID],
                    in1=exs[:].broadcast_to([P, wc * 4, 32]), op=OP.mult,
                )

                for (s0, nw, kb, cb) in chunks[tag]:
                    # one batched gather for the whole chunk: padded src rows
                    g = gp.tile([P, KMAX * ROWP], bf16, tag="g")
                    g3 = g.rearrange("p (k r) -> p k r", r=ROWP)
                    nc.gpsimd.dma_gather(
                        out_ap=g3[:, :kb, :], in_ap=table_full[tag][:, :],
                        idxs_ap=gpk[:, cb * 8 : (cb + kb) * 8],
                        num_idxs=kb * P, num_idxs_reg=kb * P, elem_size=ROWP,
                        single_packet=False,
                    )
                    # adn[dst] per edge, on-chip: replicate dst ids across
                    # partitions (ones-outer-product), indicator vs iota_col,
                    # then one small matmul per block gathers adn_win rows.
                    dstT = dtp.tile([1, KMAX * P], bf16, tag="dstT")
                    nc.sync.dma_start(
                        out=dstT[:, : kb * P],
                        in_=dstlocT_d[0:1, cb * P : (cb + kb) * P],
                    )
                    mt = mtp.tile([P, KMAX * P], bf16, tag="mt")
                    for c0m in range(0, kb * P, 512):
                        cwm = min(512, kb * P - c0m)
                        bc = psA.tile([P, 512], f32, space="PSUM", tag="a", name="bc")
                        nc.tensor.matmul(
                            bc[:, :cwm], lhsT=ones1[:, :],
                            rhs=dstT[0:1, c0m : c0m + cwm], start=True, stop=True,
                        )
                        nc.vector.tensor_scalar(
                            out=mt[:, c0m : c0m + cwm], in0=bc[:, :cwm],
                            scalar1=iota_col[:, 0:1], scalar2=None, op0=OP.is_equal,
                        )
                    a_sb = ap_.tile([P, KMAX * 4], bf16, tag="a")
                    kq0 = 0
                    for wl in range(nw):
                        s = s0 + wl
                        nb = nbl[s]
                        if nb == 0:
                            continue
                        adn_ps = psD.tile([P, KMAX * 4], f32, space="PSUM", tag="d", name="adn")
                        for b in range(nb):
                            nc.tensor.matmul(
                                adn_ps[:, b * 4 : (b + 1) * 4],
                                lhsT=mt[:, (kq0 + b) * P : (kq0 + b + 1) * P],
                                rhs=rows3[:, s, HID + 4 : ROW],
                                start=True, stop=True,
                            )
                        nc.vector.tensor_copy(
                            out=a_sb[:, kq0 * 4 : (kq0 + nb) * 4],
                            in_=adn_ps[:, : nb * 4],
                        )
                        kq0 += nb
                    # z = asn_src + adn_dst ; leaky relu ; exp (-> in place of asn)
                    z = zp.tile([P, KMAX * 4], f32, tag="z")
                    nc.vector.tensor_add(
                        out=z[:, : kb * 4], in0=g3[:, :kb, HID:GROW], in1=a_sb[:, : kb * 4]
                    )
                    z2 = zp.tile([P, KMAX * 4], f32, tag="z2")
                    nc.vector.tensor_scalar_mul(z2[:, : kb * 4], z[:, : kb * 4], 0.2)
                    nc.vector.tensor_tensor(
                        out=z[:, : kb * 4], in0=z[:, : kb * 4], in1=z2[:, : kb * 4],
                        op=OP.max,
                    )
                    ex = zp.tile([P, KMAX * 4], bf16, tag="ex")
                    nc.scalar.activation(ex[:, : kb * 4], z[:, : kb * 4], AF.Exp)
                    nc.vector.tensor_copy(out=g3[:, :kb, HID:GROW], in_=ex[:, : kb * 4])
                    nc.vector.tensor_tensor(
                        out=g3[:, :kb, 0:HID], in0=g3[:, :kb, 0:HID],
                        in1=ex[:, : kb * 4].broadcast_to([P, kb * 4, 32]), op=OP.mult,
                    )
                    # destination indicators for every block of the chunk
                    msb = mp.tile([P, KMAX * P], bf16, tag="msb")
                    nc.vector.tensor_tensor(
                        out=msb[:, : kb * P], in0=iota_tiled[:, : kb * P],
                        in1=dstloc[:, cb : cb + kb].broadcast_to([P, kb, P]),
                        op=OP.is_equal,
                    )
                    # scatter-accumulate per window
                    acc = psAcc.tile([P, WMAX * GROW], f32, space="PSUM", tag="acc")
                    kq = 0
                    for wl in range(nw):
                        s = s0 + wl
                        av = acc[:, wl * GROW : (wl + 1) * GROW]
                        nc.tensor.matmul(
                            av, lhsT=ident_bf[:, :],
                            rhs=ues[:, s * GROW : (s + 1) * GROW],
                            start=True, stop=(nbl[s] == 0),
                        )
                        for b in range(nbl[s]):
                            nc.tensor.matmul(
                                av,
                                lhsT=msb[:, (kq + b) * P : (kq + b + 1) * P],
                                rhs=g[:, (kq + b) * ROWP : (kq + b) * ROWP + GROW],
                                start=False, stop=(b == nbl[s] - 1),
                            )
                        kq += nbl[s]
                    # epilogue, batched over the chunk's windows
                    acc3 = acc.rearrange("p (w r) -> p w r", r=GROW)
                    ssb = zp.tile([P, WMAX * 4], f32, tag="ssb")
                    nc.vector.tensor_scalar_add(
                        ssb[:, : nw * 4], acc3[:, :nw, HID:GROW], 1e-16
                    )
                    rsb = zp.tile([P, WMAX * 4], f32, tag="rsb")
                    nc.vector.reciprocal(rsb[:, : nw * 4], ssb[:, : nw * 4])
                    hw = hwp.tile([P, WMAX * P], f32, tag="hw")
                    nc.vector.tensor_tensor(
                        out=hw[:, : nw * P], in0=acc3[:, :nw, 0:HID],
                        in1=rsb[:, : nw * 4].broadcast_to([P, nw * 4, 32]), op=OP.mult,
                    )
                    nc.vector.tensor_scalar(
                        out=hw[:, : nw * P], in0=hw[:, : nw * P],
                        scalar1=consts[f"{tag}_bias"][:, layer],
                        scalar2=0.0, op0=OP.add, op1=OP.max,
                    )
                    for wl in range(nw):
                        s = s0 + wl
                        hws = hw[:, wl * P : (wl + 1) * P]
                        if last:
                            nc.tensor.matmul(
                                pool_ps[tag][:, :], lhsT=hws,
                                rhs=consts[f"{tag}_ind"][:, s * G : (s + 1) * G],
                                start=(s == 0), stop=(s == wc - 1),
                            )
                        else:
                            htp = psB.tile([P, P], f32, space="PSUM", tag="b", name="htp")
                            nc.tensor.transpose(htp[:, :], hws, ident[:, :])
                            nc.vector.tensor_copy(
                                out=hT[tag][:, s * P : (s + 1) * P], in_=htp[:, :]
                            )

            # pipeline: AG(tag, i) overlaps the other branch's edge phase
            build_table("p", 0)
            build_table("l", 0)
            for layer in range(DEPTH):
                edge_phase("p", layer)
                if layer < DEPTH - 1:
                    build_table("p", layer + 1)
                edge_phase("l", layer)
                if layer < DEPTH - 1:
                    build_table("l", layer + 1)

            pool_sb = hwp.tile([P, 2 * G], f32, tag="poolsb")
            nc.vector.tensor_copy(out=pool_sb[:, 0:G], in_=pool_ps["p"][:, :])
            nc.vector.tensor_copy(out=pool_sb[:, G : 2 * G], in_=pool_ps["l"][:, :])
            nc.sync.dma_start(out=pool_in[:, :], in_=pool_sb[:])
            nc.gpsimd.collective_compute(
                "AllReduce", mybir.AluOpType.add,
                replica_groups=[list(range(N_CORES))],
                ins=[pool_in[:, :]], outs=[pool_out[:, :]],
            )
            jt = hwp.tile([P, 2 * G], f32, tag="jt")
            nc.sync.dma_start(out=jt[:], in_=pool_out[:, :])
            nc.vector.tensor_tensor(out=jt[:], in0=jt[:], in1=recip_cnt[:, :], op=OP.mult)

            # h1 = relu(W1 @ j + b1): two 128-halves
            h1 = []
            for half in range(2):
                ps = psPool.tile([P, G], f32, space="PSUM", tag=f"pp{'p' if half else 'l'}", name=f"mlp{half}")
                nc.tensor.matmul(ps[:, :], lhsT=mlpW[:, 2 * half], rhs=jt[:, 0:G],
                                 start=True, stop=False)
                nc.tensor.matmul(ps[:, :], lhsT=mlpW[:, 2 * half + 1], rhs=jt[:, G : 2 * G],
                                 start=False, stop=True)
                t = zp.tile([P, G], f32, tag=f"h1{half}")
                nc.vector.tensor_scalar(
                    out=t[:], in0=ps[:, :], scalar1=mlpb[:, half],
                    scalar2=0.0, op0=OP.add, op1=OP.max,
                )
                h1.append(t)
            ps = psPool.tile([P, G], f32, space="PSUM", tag="ppp", name="mlp2")
            nc.tensor.matmul(ps[:, :], lhsT=mlpW[:, 4], rhs=h1[0][:], start=True, stop=False)
            nc.tensor.matmul(ps[:, :], lhsT=mlpW[:, 5], rhs=h1[1][:], start=False, stop=True)
            h2 = zp.tile([P, G], f32, tag="h2")
            nc.vector.tensor_scalar(
                out=h2[:], in0=ps[:, :], scalar1=mlpb[:, 2],
                scalar2=0.0, op0=OP.add, op1=OP.max,
            )
            pps = psC.tile([1, G], f32, space="PSUM", tag="c", name="predps")
            nc.tensor.matmul(pps[:, :], lhsT=w3col[:], rhs=h2[:], start=True, stop=True)
            pred = zp.tile([1, G], f32, tag="predsb")
            nc.vector.tensor_scalar_add(pred[:], pps[:, :], meta["b3"])
            nc.sync.dma_start(out=out[0:1, :], in_=pred[:])

    nc.compile()
    return nc


# ----------------------------------------------------------------------------
# entry point
# ----------------------------------------------------------------------------

LAST_EXEC_NS = None
LAST_RESULT = None


def kernel(_trace=False, **inputs) -> np.ndarray:
    global LAST_EXEC_NS, LAST_RESULT
    from concourse.bass_utils import run_bass_kernel_spmd

    meta, in_maps = _prep(inputs)
    nc = _build(meta)
    res = run_bass_kernel_spmd(nc, in_maps, list(range(N_CORES)), trace=_trace)
    LAST_EXEC_NS = res.exec_time_ns
    LAST_RESULT = res
    return np.asarray(res.results[0]["out"], np.float32).reshape(G)
